# revision 24
# baseline (speedup 1.0000x reference)
"""Trainium2 Bass kernel for CLIP + CMP loss (nn_CLIPWithCMPLoss), fp8 version.

Full-input contract: kernel(**inputs) takes the complete arrays and returns the
scalar loss. Batch rows are sharded across 8 NeuronCores; each core computes
512 rows of the [B, B] logits matrix (softmax rows fully local) and emits
per-row statistics {masked-softmax block sums, target prob, masked-denom}
which the host combines into the scalar loss. The text encoder is recomputed
per core (collectives here cost more than the PE time they would save).

All matmuls are float8_e4m3 with MatmulPerfMode.DoubleRow (k-tile pairs,
256-deep contraction per instruction) — ~2-3x the bf16 PE rate. PSUM and
stats are f32.

Normalization is folded into the INPUTS on the host (linearity of the
encoders): texts_j *= ST/||txt_emb_j||, images_i *= SI/||img_emb_i||, weights
*= SW, so the device embeddings come out pre-normalized (no per-column
normalize pass) and the logits scale is the constant esc/(ST*SI*SW^2) applied
inside the Exp activation.

The pairwise label mask is folded into the LOGITS MATMUL: labels are hashed
to 256 classes; one extra DoubleRow pair per 512-col block contracts
(-240*onehot_hash(row)) x (240*onehot_hash(col)), planting ~-146 in the
logit wherever hash classes collide. The Exp then directly yields
m1 = E*[diff-label] (masked cols underflow to ~e-140), the Exp accum gives
s ~= sum(m1) (~0.5% low, negligible in log s), and the only remaining DVE
work is the Et one-hot gather and the Sm threshold-sum STTs. Rows whose
target column t=labels[i] would be masked (hash(labels[t]) == hash(labels[i]),
~20 of 4096) get their row-onehot zeroed on the host: those rows run fully
unmasked, keeping Et and s exact there (their Sm then includes the ~4
same-label cols — noise in a ~2000-term denominator).

The whole kernel is a single software pipeline over the 8 text column
blocks: encode block n (6 DR pairs per e-tile), then immediately run all 4
row-tiles' logits (2 main DR + 1 onehot DR each), Exp (ACT, accum -> s
block-sum), per-block Et gathers, and per-block Sm STTs, so PE, ACT and DVE
stay concurrently busy from ~10us on and the post-matmul tail is one block's
worth of DVE work. Sm for blocks 0-1 defers until Et = Et_a + Et_b is
complete, interleaved into blocks 2-3.

Per row i (t = labels[i], esc = exp(logit_scale)):
  m1_ij = E_ij * [hash-diff]     (from the masked-exp)
  s_i   = sum_j m1_ij            (~= softmax denominator)
  Et_i  = m1[i, t]               (exact: row unmasked if t would collide)
  Sm_i  = sum_j m1 * [m1 > Et]
  loss = mean_i (log s_i - log Et_i) + sum_i [Sm_i>0] * Et_i/(Sm_i + EPS*s_i) / B
"""

import sys

if "/opt/trn_rl_repo" not in sys.path:
    sys.path.insert(0, "/opt/trn_rl_repo")

import numpy as np

B = 4096
D = 768
E = 512
P = 128
NCORES = 8
SHARD = B // NCORES          # 512 rows per core
RT = SHARD // P              # 4 row-tiles per core
KD = D // P                  # 6 contraction tiles for the encoders
KE = E // P                  # 4 contraction tiles for the logits matmul
NBLK = B // E                # 8 column blocks
GW = 1024                    # Et gather width (labels < 1000): blocks 0-1
NCLS = 256                   # hashed label classes (2 k-tiles = 1 DR pair)
# per row-tile stats layout: s[0..7], Et_a, Et_b, Et, Sm[0..7] -> 19, pad to 20
NSTAT = 20
EPS = 1e-10

# host-side fp8 gains: texts *= ST/||txt||, images *= SI/||img||, W *= SW
ST, SI, SW = 8.0, 11.0, 8.0
OHV = 240.0                  # onehot matmul operand magnitude (fp8 e4m3 max)
ESC0 = float(np.exp(np.log(1.0 / 0.07)))  # compiled-in logit scale; deviations
                                          # of the logit_scale input fold into
                                          # the host image prescale

_CACHE = {}


def _build():
    import concourse.tile as tile
    from concourse import bacc, mybir

    f32 = mybir.dt.float32
    f16 = mybir.dt.float16
    fp8 = mybir.dt.float8e4
    AF = mybir.ActivationFunctionType
    OP = mybir.AluOpType
    DR = mybir.MatmulPerfMode.DoubleRow

    nc = bacc.Bacc("TRN2", target_bir_lowering=False, debug=False,
                   num_devices=NCORES)

    d_images = nc.dram_tensor("imagesP", [P, KD, SHARD], fp8, kind="ExternalInput").ap()
    d_texts = nc.dram_tensor("textsP", [P, NBLK, KD, E], fp8, kind="ExternalInput").ap()
    d_wimg = nc.dram_tensor("W_imgP", [P, KD, E], fp8, kind="ExternalInput").ap()
    d_wtxt = nc.dram_tensor("W_txtP", [P, KD, E], fp8, kind="ExternalInput").ap()
    d_ohcol = nc.dram_tensor("ohcolT", [P, 2, B], fp8, kind="ExternalInput").ap()
    d_ohrow = nc.dram_tensor("ohrowT", [P, 2, SHARD], fp8, kind="ExternalInput").ap()
    d_iota = nc.dram_tensor("iotab", [P, GW], f16, kind="ExternalInput").ap()
    d_labrow = nc.dram_tensor("labrow", [P, RT], f32, kind="ExternalInput").ap()
    d_zeros = nc.dram_tensor("zeros8", [P, 2, E + P], fp8, kind="ExternalInput").ap()
    d_stats = nc.dram_tensor("stats", [P, RT * NSTAT], f32, kind="ExternalOutput").ap()

    escale = float(ESC0 / (ST * SI * SW * SW))

    with tile.TileContext(nc) as tc:
        with tc.tile_pool(name="sb", bufs=1) as sb, \
             tc.tile_pool(name="scrp", bufs=4) as scrp, \
             tc.tile_pool(name="warmps", bufs=1, space="PSUM") as warmps, \
             tc.tile_pool(name="encps", bufs=3, space="PSUM") as encps, \
             tc.tile_pool(name="psL", bufs=4, space="PSUM") as psL:

            iota_sb = sb.tile([P, GW], f16)
            labrow_sb = sb.tile([P, RT], f32)
            ohcol_sb = sb.tile([P, 2, B], fp8)
            ohrow_sb = sb.tile([P, 2, SHARD], fp8)

            imgT = sb.tile([P, KE, SHARD], fp8)       # img embT (lhsT), prenormalized
            txtT = sb.tile([P, KE, B], fp8)           # txt embT (rhs), prenormalized
            m1 = sb.tile([P, RT, B], fp8)             # masked exp(logits)
            stats_sb = sb.tile([P, RT * NSTAT], f32)

            # PE warmup on zeros: keeps the activity monitor busy from t~0 so
            # real matmuls run ramped, not at the cold half clock.
            wz = sb.tile([P, 2, P], fp8)
            nc.gpsimd.memset(wz[:], 0.0)
            wrhs = sb.tile([P, 2, E], fp8)
            nc.gpsimd.memset(wrhs[:], 0.0)
            wps = warmps.tile([P, E], f32)
            for w in range(24):
                nc.tensor.matmul(wps[:], wz[:], wrhs[:],
                                 start=(w == 0), stop=(w == 23), perf_mode=DR)

            # All compute-critical inputs ride ONE queue (sync) in exact
            # consumption order — the DMA engines drain multiple queues
            # concurrently, so spreading across queues lets later transfers
            # steal bandwidth from the block the PE needs next. Only the small
            # mask/gather operands (needed from the first logits block) go on
            # the scalar queue in parallel.
            wtxt_sb = sb.tile([P, KD, E], fp8)
            nc.sync.dma_start(wtxt_sb[:], d_wtxt)
            texts_sb = sb.tile([P, NBLK, KD, E], fp8)
            nc.sync.dma_start(texts_sb[:, 0], d_texts[:, 0])
            wimg_sb = sb.tile([P, KD, E], fp8)
            nc.sync.dma_start(wimg_sb[:], d_wimg)
            images_sb = sb.tile([P, KD, SHARD], fp8)
            nc.sync.dma_start(images_sb[:], d_images)
            for n in range(1, NBLK):
                nc.sync.dma_start(texts_sb[:, n], d_texts[:, n])
            nc.scalar.dma_start(ohrow_sb[:], d_ohrow)
            nc.scalar.dma_start(ohcol_sb[:], d_ohcol)
            nc.scalar.dma_start(iota_sb[:], d_iota)
            nc.scalar.dma_start(labrow_sb[:], d_labrow)

            def encode_block(n):
                x_sb = texts_sb[:, n]
                cols = slice(n * E, (n + 1) * E)
                for m in range(KE):
                    enc = encps.tile([P, E], f32, tag="enc")
                    for kp in range(KD // 2):
                        nc.tensor.matmul(
                            enc[:],
                            wtxt_sb[:, 2 * kp:2 * kp + 2, m * P:(m + 1) * P],
                            x_sb[:, 2 * kp:2 * kp + 2, :],
                            start=(kp == 0), stop=(kp == KD // 2 - 1),
                            perf_mode=DR)
                    if m % 2 == 0:
                        nc.vector.tensor_copy(txtT[:, m, cols], enc[:])
                    else:
                        nc.scalar.activation(txtT[:, m, cols], enc[:], AF.Copy)

            # text block 0 encodes first (its inputs lead the DMA stream), the
            # image encoder hides the remaining transfer latency
            encode_block(0)
            for m in range(KE):
                enc = encps.tile([P, E], f32, tag="enc")
                for kp in range(KD // 2):
                    nc.tensor.matmul(
                        enc[:],
                        wimg_sb[:, 2 * kp:2 * kp + 2, m * P:(m + 1) * P],
                        images_sb[:, 2 * kp:2 * kp + 2, :],
                        start=(kp == 0), stop=(kp == KD // 2 - 1), perf_mode=DR)
                if m % 2 == 0:
                    nc.vector.tensor_copy(imgT[:, m, :], enc[:])
                else:
                    nc.scalar.activation(imgT[:, m, :], enc[:], AF.Copy)

            # --- fused logits/loss + next-block-encoder pipeline ---
            deferred = []                      # Sm (t, n) waiting on Et
            for n in range(NBLK):
                cols = slice(n * E, (n + 1) * E)
                for t in range(RT):
                    base = t * NSTAT
                    rows = slice(t * P, (t + 1) * P)
                    ps = psL.tile([P, E], f32, tag="L")
                    for kp in range(KE // 2):
                        nc.tensor.matmul(
                            ps[:], imgT[:, 2 * kp:2 * kp + 2, rows],
                            txtT[:, 2 * kp:2 * kp + 2, cols],
                            start=(kp == 0), stop=False, perf_mode=DR)
                    # hashed-label mask: plants ~-146 on same-class cols
                    nc.tensor.matmul(
                        ps[:], ohrow_sb[:, :, rows], ohcol_sb[:, :, cols],
                        start=False, stop=True, perf_mode=DR)
                    # masked exp -> m1 block, accum -> s block-sum
                    nc.scalar.activation(
                        m1[:, t, cols], ps[:], AF.Exp, scale=escale,
                        accum_out=stats_sb[:, base + n:base + n + 1])
                    if n < 2:
                        # Et partial gather over this block's 512 cols
                        scr = scrp.tile([P, E], f16, tag="scr")
                        nc.vector.scalar_tensor_tensor(
                            scr[:], iota_sb[:, n * E:(n + 1) * E],
                            labrow_sb[:, t:t + 1], m1[:, t, cols],
                            op0=OP.is_equal, op1=OP.mult,
                            accum_out=stats_sb[:, base + 8 + n:base + 9 + n])
                        deferred.append((t, n))
                        if n == 1:
                            # Et = Et_a + Et_b (tiny DVE add keeps ordering)
                            nc.vector.tensor_tensor(
                                stats_sb[:, base + 10:base + 11],
                                stats_sb[:, base + 8:base + 9],
                                stats_sb[:, base + 9:base + 10], OP.add)
                    else:
                        todo = [(t, n)]
                        if deferred and n < 4:
                            todo.append(deferred.pop(0))
                        for tt, nn in todo:
                            bb = tt * NSTAT
                            ccols = slice(nn * E, (nn + 1) * E)
                            et_col = stats_sb[:, bb + 10:bb + 11]
                            m2 = scrp.tile([P, E], fp8, tag="m2")
                            nc.vector.scalar_tensor_tensor(
                                m2[:], m1[:, tt, ccols], et_col,
                                m1[:, tt, ccols],
                                op0=OP.is_gt, op1=OP.mult,
                                accum_out=stats_sb[:, bb + 11 + nn:bb + 12 + nn])
                if n + 1 < NBLK:
                    encode_block(n + 1)

            for tt, nn in deferred:
                bb = tt * NSTAT
                ccols = slice(nn * E, (nn + 1) * E)
                et_col = stats_sb[:, bb + 10:bb + 11]
                m2 = scrp.tile([P, E], fp8, tag="m2")
                nc.vector.scalar_tensor_tensor(
                    m2[:], m1[:, tt, ccols], et_col, m1[:, tt, ccols],
                    op0=OP.is_gt, op1=OP.mult,
                    accum_out=stats_sb[:, bb + 11 + nn:bb + 12 + nn])

            nc.sync.dma_start(d_stats, stats_sb[:])

    nc.compile()
    return nc


def _to_fp8(x):
    import ml_dtypes
    return np.ascontiguousarray(x, np.float32).astype(ml_dtypes.float8_e4m3)


def _ki_ko(x):
    """[K_total, X] -> [P, K_total//P, X] with K split as (ko ki)->ki ko."""
    kt = x.shape[0]
    return np.ascontiguousarray(
        x.reshape(kt // P, P, *x.shape[1:]).transpose(1, 0, *range(2, x.ndim + 1)))


def _in_maps(images, texts, labels, W_img, W_txt, logit_scale):
    ls = float(logit_scale)

    # fp8 operand emulation on host (f32 BLAS on the rounded operands) to get
    # norms matching what the device computes
    img8 = _to_fp8(images).astype(np.float32)
    txt8 = _to_fp8(texts).astype(np.float32)
    w_img8 = _to_fp8(W_img * SW).astype(np.float32)
    w_txt8 = _to_fp8(W_txt * SW).astype(np.float32)
    n_img = np.linalg.norm(img8 @ w_img8, axis=1) / SW
    n_txt = np.linalg.norm(txt8 @ w_txt8, axis=1) / SW

    si_eff = SI * float(np.exp(ls)) / ESC0
    texts_n = _to_fp8(texts * (ST / n_txt)[:, None]).astype(np.float32)
    images_n = _to_fp8(images * (si_eff / n_img)[:, None]).astype(np.float32)

    # device layouts
    textsT = texts_n.T                                   # [D, B]
    textsP = _to_fp8(np.ascontiguousarray(
        textsT.reshape(KD, P, NBLK, E).transpose(1, 2, 0, 3)))
    w_txtP = _to_fp8(_ki_ko(w_txt8))
    w_imgP = _to_fp8(_ki_ko(w_img8))

    # hashed-class onehots for the in-matmul label mask
    hcls = (labels % NCLS).astype(np.int64)              # [B]
    ohcol = np.zeros((NCLS, B), np.float32)
    ohcol[hcls, np.arange(B)] = OHV
    ohcolT = _to_fp8(ohcol.reshape(2, P, B).transpose(1, 0, 2))

    # rows whose target column would be masked run unmasked (Et, s exact)
    tcol = labels.astype(np.int64)                       # target col = label
    unmask = hcls[tcol] == hcls                          # [B]

    iotab = np.ascontiguousarray(
        np.broadcast_to(np.arange(GW, dtype=np.float16), (P, GW)))
    lab_f = labels.astype(np.float32)

    maps = []
    for c in range(NCORES):
        sl = slice(c * SHARD, (c + 1) * SHARD)
        ohrow = np.zeros((NCLS, SHARD), np.float32)
        keep = ~unmask[sl]
        ohrow[hcls[sl][keep], np.arange(SHARD)[keep]] = -OHV
        ohrowT = _to_fp8(ohrow.reshape(2, P, SHARD).transpose(1, 0, 2))
        imagesP = _to_fp8(_ki_ko(
            np.ascontiguousarray(images_n.T[:, sl])))
        maps.append({
            "imagesP": imagesP,
            "textsP": textsP,
            "W_imgP": w_imgP,
            "W_txtP": w_txtP,
            "ohcolT": ohcolT,
            "ohrowT": ohrowT,
            "iotab": iotab,
            "labrow": np.ascontiguousarray(lab_f[sl].reshape(RT, P).T),
            "zeros8": _to_fp8(np.zeros((P, 2, E + P), np.float32)),
        })
    return maps


def _assemble(stats_list):
    """Combine the 8 cores' [P, RT*NSTAT] stats into the scalar loss (f64)."""
    clip_sum = 0.0
    cmp_sum = 0.0
    for arr in stats_list:
        a = arr.reshape(P, RT, NSTAT).astype(np.float64)
        s = a[:, :, 0:NBLK].sum(axis=2)
        et = a[:, :, 10]
        sm = a[:, :, 11:11 + NBLK].sum(axis=2)
        clip_sum += float(np.sum(np.log(s) - np.log(et)))
        cmp_sum += float(np.sum(np.where(sm > 0.0, et / (sm + EPS * s), 0.0)))
    return np.float32(clip_sum / B + cmp_sum / B)


def kernel(images, texts, labels, W_img, W_txt, logit_scale):
    from concourse import bass_utils

    images = np.asarray(images, np.float32)
    texts = np.asarray(texts, np.float32)
    labels = np.asarray(labels)
    W_img = np.asarray(W_img, np.float32)
    W_txt = np.asarray(W_txt, np.float32)

    assert int(labels.max()) < GW, "labels must fit the Et gather width"
    if 0 not in _CACHE:
        _CACHE[0] = _build()
    nc = _CACHE[0]

    maps = _in_maps(images, texts, labels, W_img, W_txt, logit_scale)
    res = bass_utils.run_bass_kernel_spmd(nc, maps, core_ids=list(range(NCORES)))
    return _assemble([res.results[c]["stats"] for c in range(NCORES)])
